# revision 12
# baseline (speedup 1.0000x reference)
"""Expert-parallel MoE (top-1 routing) on 8 TRN2 NeuronCores.

Strategy (per sharding hint): one expert per core. The host computes the
top-1 gate assignment (tiny [N,8] matmul) to *dispatch* tokens: tokens are
gathered per expert, transposed to feature-major [D, C] and zero-padded to
a common capacity C so all 8 cores run one SPMD program. Each core then:
  - recomputes gate logits/softmax for its tokens on-device (fp32 matmul)
    to get the chosen-prob scale row and the per-expert prob sums that feed
    the balancing loss,
  - runs its expert's FFN in bf16: h = gelu(x @ w1 + b1) [feature-major so
    both matmuls consume the weights in natural layout as the stationary
    operand and no transposes are needed], y = (h @ w2 + b2) * chosen_prob,
    with b2 folded in as a rank-1 (K=1) PSUM accumulation,
  - writes y [D, C] and probsum [8].
The host scatters the per-expert outputs back to token order and reduces
the 8x8 probsums into the scalar balancing loss.
"""

import math

import numpy as np
import ml_dtypes

import concourse.bass as bass
import concourse.bass_isa as bass_isa
import concourse.mybir as mybir
import concourse.tile as tile
from concourse.bass_utils import run_bass_kernel_spmd

P = 128
D = 768
F = 3072
E = 8
KD = D // P   # 6
KF = F // P   # 24
BF16 = mybir.dt.bfloat16
F32 = mybir.dt.float32
AF = mybir.ActivationFunctionType


def _split_multi_waits(nc, maxw=1):
    """This container's walrus rejects >1 semaphore wait per instruction
    ("Too many sync wait commands" on the Tile tail drain). Move extra
    waits onto preceding NoOps on the same engine; engine queues execute
    in order so blocking semantics are preserved."""
    for f in nc.m.functions:
        for bb in f.blocks:
            out = []
            changed = False
            for inst in bb.instructions:
                si = inst.sync_info
                if si is not None and si.on_wait and len(si.on_wait) > maxw:
                    waits = list(si.on_wait)
                    for j, w in enumerate(waits[:-maxw]):
                        nop = mybir.InstEventSemaphore(
                            name=f"Wsplit-{inst.name}-{j}", ins=[], outs=[])
                        nop.engine = inst.engine
                        nop.sync_info = mybir.SyncInfo(on_wait=[w], on_update=[])
                        out.append(nop)
                    si.on_wait = waits[-maxw:]
                    changed = True
                out.append(inst)
            if changed:
                bb.instructions = out


def _chunks(C):
    out = []
    n0 = 0
    while n0 < C:
        out.append((n0, min(512, C - n0)))
        n0 += 512
    return out


def build_nc(C):
    TT = C // P
    nc = bass.Bass()

    xTf = nc.declare_dram_parameter("xTf", [D, C], F32, isOutput=False)
    xTb = nc.declare_dram_parameter("xTb", [D, C], BF16, isOutput=False)
    w1 = nc.declare_dram_parameter("w1", [D, F], BF16, isOutput=False)
    w2 = nc.declare_dram_parameter("w2", [F, D], BF16, isOutput=False)
    b1m = nc.declare_dram_parameter("b1m", [P, KF], F32, isOutput=False)
    b2r = nc.declare_dram_parameter("b2r", [1, D], BF16, isOutput=False)
    gwT = nc.declare_dram_parameter("gwT", [D, E], F32, isOutput=False)
    maskc = nc.declare_dram_parameter("maskc", [P, C // P], F32, isOutput=False)
    ident = nc.declare_dram_parameter("ident", [P, P], F32, isOutput=False)
    ones1x128 = nc.declare_dram_parameter("ones1x128", [1, P], F32, isOutput=False)
    onesrow = nc.declare_dram_parameter("onesrow", [1, C], BF16, isOutput=False)

    out = nc.declare_dram_parameter("out", [D, C], F32, isOutput=True)
    stats = nc.declare_dram_parameter("stats", [E, 1], F32, isOutput=True)

    xTf_t = xTf.rearrange("(k p) c -> k p c", p=P)
    xTb_t = xTb.rearrange("(k p) c -> k p c", p=P)
    w1_t = w1.rearrange("(k p) f -> k p f", p=P)
    w2_t = w2.rearrange("(k p) d -> k p d", p=P)
    gwT_t = gwT.rearrange("(k p) e -> k p e", p=P)

    with tile.TileContext(nc) as tc:
        with (
            tc.tile_pool(name="const", bufs=1) as const,
            tc.tile_pool(name="xin", bufs=1) as xin,
            tc.tile_pool(name="wts", bufs=1) as wts,
            tc.tile_pool(name="hbuf", bufs=1) as hbuf,
            tc.tile_pool(name="gate", bufs=2) as gate,
            tc.tile_pool(name="evict", bufs=3) as evict,
            tc.tile_pool(name="psg", bufs=1, space="PSUM") as psg,
            tc.tile_pool(name="psmm", bufs=4, space="PSUM") as psmm,
        ):
            # ---- constants / inputs to SBUF ----
            b1_sb = const.tile([P, KF], F32, tag="b1", name="b1")
            nc.sync.dma_start(b1_sb[:], b1m[:, :])
            b2_sb = const.tile([1, D], BF16, tag="b2", name="b2")
            nc.sync.dma_start(b2_sb[:], b2r[:, :])
            mask_sb = const.tile([P, C // P], F32, tag="mask", name="mask")
            nc.sync.dma_start(mask_sb[:], maskc[:, :])
            ident_sb = const.tile([P, P], F32, tag="ident", name="ident")
            nc.sync.dma_start(ident_sb[:], ident[:, :])
            ones1x128_sb = const.tile([1, P], F32, tag="o1p", name="o1p")
            nc.sync.dma_start(ones1x128_sb[:], ones1x128[:, :])
            onesrow_sb = const.tile([1, C], BF16, tag="orow", name="orow")
            nc.sync.dma_start(onesrow_sb[:], onesrow[:, :])

            gw_sb = []
            for k in range(KD):
                t = const.tile([P, E], F32, tag=f"gw{k}", name=f"gw{k}")
                nc.sync.dma_start(t[:], gwT_t[k])
                gw_sb.append(t)

            xf_sb = []
            xb_sb = []
            for k in range(KD):
                t = xin.tile([P, C], F32, tag=f"xf{k}", name=f"xf{k}")
                nc.sync.dma_start(t[:], xTf_t[k])
                xf_sb.append(t)
                tb = xin.tile([P, C], BF16, tag=f"xb{k}", name=f"xb{k}")
                nc.sync.dma_start(tb[:], xTb_t[k])
                xb_sb.append(tb)

            w1_sb = []
            for k in range(KD):
                t = wts.tile([P, F], BF16, tag=f"w1_{k}", name=f"w1_{k}")
                nc.sync.dma_start(t[:], w1_t[k])
                w1_sb.append(t)
            w2_sb = []
            for k in range(KF):
                t = wts.tile([P, D], BF16, tag=f"w2_{k}", name=f"w2_{k}")
                nc.sync.dma_start(t[:], w2_t[k])
                w2_sb.append(t)

            # ---- gate pass (fp32), token-major [128 tokens, 8 experts] ----
            # chosen prob = 1/sum(exp(l - max)); transposed to a row with a
            # plain fp32 matmul against the identity, then broadcast to all
            # 128 partitions with a ones (x) row matmul.
            pb_sb = gate.tile([P, C], F32, tag="pb", name="pb")
            st_ps = psg.tile([E, 1], F32, tag="st", name="st")
            for tt in range(TT):
                sl = slice(tt * P, (tt + 1) * P)
                lg = psg.tile([P, E], F32, tag="lg", name="lg")
                for k in range(KD):
                    nc.tensor.matmul(lg[:], xf_sb[k][:, sl], gw_sb[k][:],
                                     start=(k == 0), stop=(k == KD - 1))
                mx = gate.tile([P, 1], F32, tag="mx", name="mx")
                nc.vector.reduce_max(mx[:], lg[:], axis=mybir.AxisListType.X)
                negm = gate.tile([P, 1], F32, tag="negm", name="negm")
                nc.vector.tensor_scalar_mul(negm[:], mx[:], -1.0)
                pe_t = gate.tile([P, E], F32, tag="pe", name="pe")
                sume = gate.tile([P, 1], F32, tag="sume", name="sume")
                nc.scalar.activation(pe_t[:], lg[:], AF.Exp, bias=negm[:],
                                     accum_out=sume[:])
                rc = gate.tile([P, 1], F32, tag="rc", name="rc")
                nc.vector.reciprocal(rc[:], sume[:])
                probs = gate.tile([P, E], F32, tag="probs", name="probs")
                nc.vector.tensor_scalar_mul(probs[:], pe_t[:], rc[:])
                nc.tensor.matmul(st_ps[:], probs[:], mask_sb[:, tt:tt + 1],
                                 start=(tt == 0), stop=(tt == TT - 1))
                ct_ps = psg.tile([1, P], F32, tag="ct", name="ct")
                nc.tensor.matmul(ct_ps[:], rc[:], ident_sb[:],
                                 start=True, stop=True)
                ct_sb = gate.tile([1, P], F32, tag="ctsb", name="ctsb")
                nc.vector.tensor_copy(ct_sb[:], ct_ps[:])
                pb_ps = psg.tile([P, P], F32, tag="pbps", name="pbps")
                nc.tensor.matmul(pb_ps[:], ones1x128_sb[:], ct_sb[:],
                                 start=True, stop=True)
                nc.vector.tensor_copy(pb_sb[:, sl], pb_ps[:])
            stats_sb = gate.tile([E, 1], F32, tag="stacc", name="stacc")
            nc.vector.tensor_copy(stats_sb[:], st_ps[:])
            nc.sync.dma_start(stats[:, :], stats_sb[:])

            # ---- FFN ----
            h_sb = [hbuf.tile([P, C], BF16, tag=f"h{ft}", name=f"h{ft}") for ft in range(KF)]
            for (n0, nsz) in _chunks(C):
                cs = slice(n0, n0 + nsz)
                for ft in range(KF):
                    hp = psmm.tile([P, 512], F32, tag="mm", name="mm")
                    for k in range(KD):
                        nc.tensor.matmul(hp[:, :nsz],
                                         w1_sb[k][:, ft * P:(ft + 1) * P],
                                         xb_sb[k][:, cs],
                                         start=(k == 0), stop=(k == KD - 1))
                    nc.scalar.activation(h_sb[ft][:, cs], hp[:, :nsz], AF.Gelu,
                                         bias=b1_sb[:, ft:ft + 1])
            for (n0, nsz) in _chunks(C):
                cs = slice(n0, n0 + nsz)
                for dt in range(KD):
                    yp = psmm.tile([P, 512], F32, tag="mm", name="mm")
                    for k in range(KF):
                        nc.tensor.matmul(yp[:, :nsz],
                                         w2_sb[k][:, dt * P:(dt + 1) * P],
                                         h_sb[k][:, cs],
                                         start=(k == 0), stop=False)
                    nc.tensor.matmul(yp[:, :nsz],
                                     b2_sb[0:1, dt * P:(dt + 1) * P],
                                     onesrow_sb[0:1, cs],
                                     start=False, stop=True)
                    ot = evict.tile([P, 512], F32, tag="ot", name="ot")
                    nc.vector.tensor_mul(ot[:, :nsz], yp[:, :nsz], pb_sb[:, cs])
                    nc.sync.dma_start(out[dt * P:(dt + 1) * P, cs], ot[:, :nsz])

    _split_multi_waits(nc)
    return nc


_NC_CACHE = {}


def prepare(x, attention_mask, gate_w, w1, b1, w2, b2):
    """Host-side dispatch: returns (nc, in_maps, idx_per_core, counts, C)."""
    x = np.asarray(x, np.float32)
    gate_w = np.asarray(gate_w, np.float32)
    w1 = np.asarray(w1, np.float32)
    b1 = np.asarray(b1, np.float32)
    w2 = np.asarray(w2, np.float32)
    b2 = np.asarray(b2, np.float32)

    B, S, _ = x.shape
    N = B * S
    xf = x.reshape(N, D)

    # Host-side dispatch: top-1 expert per token (fp64 logits so the argmax
    # matches the fp32 reference on near-ties).
    logits = xf.astype(np.float64) @ gate_w.T.astype(np.float64)
    gate_idx = np.argmax(logits, axis=1)
    counts = np.bincount(gate_idx, minlength=E)
    order = np.argsort(gate_idx, kind="stable")
    bounds = np.concatenate([[0], np.cumsum(counts)])
    C = max(P, int(math.ceil(counts.max() / P) * P))

    if C not in _NC_CACHE:
        _NC_CACHE[C] = build_nc(C)
    nc = _NC_CACHE[C]

    gwT_np = np.ascontiguousarray(gate_w.T)
    ident_np = np.eye(P, dtype=np.float32)
    ones1x128_np = np.ones((1, P), np.float32)
    onesrow_np = np.ones((1, C), ml_dtypes.bfloat16)

    in_maps = []
    idx_per_core = []
    for c in range(E):
        idx = order[bounds[c]:bounds[c + 1]]
        idx_per_core.append(idx)
        n_c = len(idx)
        xT = np.zeros((D, C), np.float32)
        xT[:, :n_c] = xf[idx].T
        mask = np.zeros(C, np.float32)
        mask[:n_c] = 1.0
        in_maps.append({
            "xTf": xT,
            "xTb": xT.astype(ml_dtypes.bfloat16),
            "w1": w1[c].astype(ml_dtypes.bfloat16),
            "w2": w2[c].astype(ml_dtypes.bfloat16),
            "b1m": np.ascontiguousarray(b1[c].reshape(KF, P).T),
            "b2r": b2[c].reshape(1, D).astype(ml_dtypes.bfloat16),
            "gwT": gwT_np,
            "maskc": np.ascontiguousarray(mask.reshape(C // P, P).T),
            "ident": ident_np,
            "ones1x128": ones1x128_np,
            "onesrow": onesrow_np,
        })
    return nc, in_maps, idx_per_core, counts, C


def kernel(x, attention_mask, gate_w, w1, b1, w2, b2):
    x = np.asarray(x, np.float32)
    B, S, _ = x.shape
    N = B * S
    nc, in_maps, idx_per_core, counts, C = prepare(
        x, attention_mask, gate_w, w1, b1, w2, b2)

    res = run_bass_kernel_spmd(nc, in_maps, core_ids=list(range(E)))

    out_flat = np.zeros((N, D), np.float32)
    probsum = np.zeros(E, np.float64)
    for c in range(E):
        idx = idx_per_core[c]
        out_flat[idx] = res.results[c]["out"][:, :len(idx)].T
        probsum += res.results[c]["stats"].reshape(E).astype(np.float64)

    Pm = (probsum / N).astype(np.float32)
    fm = (counts / N).astype(np.float32)
    balance_loss = np.float32(E * np.sum(Pm * fm))
    gate_load = counts.astype(np.int32)
    return out_flat.reshape(B, S, D), balance_loss, gate_load
